# revision 1
# baseline (speedup 1.0000x reference)
"""Haar DWT (2x2) Trainium2 Bass kernel, v7: bf16 I/O halves HBM traffic.

Full input x: (8, 64, 512, 512) fp32. Output: tuple (ll, lh, hl, hh), each
(8, 64, 256, 256) fp32. Core i processes batch element i (pure data parallel).

The rel-err gate is 2e-2; bf16 rounding of inputs and outputs contributes
~2e-3 normalized error, so the kernel uploads x as bf16 (host astype) and
stores bf16 outputs (host upcast), halving HBM traffic per core from 128MB
to 64MB: modeled DMA floor ~186.6us instead of ~373us.

Per-core design (per 512x512 channel image):
  - ONE input DMA per channel: xc[p, rb*512 + j] = x[c, rb*128 + p, j]
    (bf16, 512 descriptors x 1KB).
  - Vertical Haar stage on PE (bf16, 1 cycle/row): per half h (row-pairs
    128h..128h+127), TWO accumulating matmuls fill psumA = 0.5*(pair sums)
    across all 128 partitions, psumB = 0.5*(pair diffs). fp32 PSUM accum.
  - Horizontal stage, balanced to keep every engine under the 186.6us floor:
      ACT: od = copy(ps odd cols) -> bf16 SBUF        (one PSUM read)
      DVE: tlow = ps even cols + od -> bf16           (one PSUM read, 1x)
      DVE: thigh = (od * 2) - tlow                     (all-SBUF bf16
           scalar_tensor_tensor, runs in the fast DVE perf mode)
    using thigh = od - ev = 2*od - (ev + od).
  - ONE output DMA per (4-channel group, output type) into fused bf16 DRAM
    tensor out4 [4, C, 256, 256]; 3-dim AP, 128 partitions, 512B descriptors
    (exactly at the >=512B full-bandwidth threshold).

Loads issue on the SP ring, stores on the ACT ring (a waiting store must not
block later loads). DMA count per core: 1 + 64 + 64 = 129.
"""

import sys

if "/opt/trn_rl_repo" not in sys.path:
    sys.path.insert(0, "/opt/trn_rl_repo")

import ml_dtypes
import numpy as np

import concourse.mybir as mybir
from concourse.bacc import Bacc
from concourse.tile import TileContext
from concourse.bass_utils import run_bass_kernel_spmd

N_CORES = 8
C = 64  # images (channels) per core
G = 4  # channels per output-store group
H = W = 512
OH = OW = 256
F32 = mybir.dt.float32
BF16 = mybir.dt.bfloat16

_cache = {}


def build_nc():
    nc = Bacc("TRN2", target_bir_lowering=False, debug=False, num_devices=N_CORES)
    x = nc.declare_dram_parameter("x", [C, H, W], BF16, isOutput=False)
    w = nc.declare_dram_parameter("w", [128, 512], BF16, isOutput=False)
    out4 = nc.declare_dram_parameter("out4", [4, C, OH, OW], BF16, isOutput=True)

    with TileContext(nc) as tc:
        with (
            tc.tile_pool(name="const", bufs=1) as cpool,
            tc.tile_pool(name="xin", bufs=6) as xpool,
            tc.tile_pool(name="outs", bufs=3) as opool,
            tc.tile_pool(name="odbuf", bufs=8) as odpool,
            tc.tile_pool(name="psum", bufs=8, space="PSUM") as ppool,
        ):
            wt = cpool.tile([128, 512], BF16)
            nc.sync.dma_start(out=wt, in_=w[:, :])
            wr = wt[:, :]
            for c0 in range(0, C, G):
                otile = {
                    k: opool.tile([128, 2 * G * OW], BF16, tag=k, name=f"o_{k}_{c0}")
                    for k in ("ll", "lh", "hl", "hh")
                }
                for gi in range(G):
                    c = c0 + gi
                    xc = xpool.tile([128, 4 * W], BF16)
                    nc.sync.dma_start(
                        out=xc[:, :].rearrange("p (r j) -> p r j", r=4),
                        in_=x[c].rearrange("(r p) j -> p r j", p=128),
                    )
                    xr = xc[:, :]
                    for h in range(2):
                        m = gi * 2 + h
                        ms = slice(m * OW, (m + 1) * OW)
                        for grp, (lo, hi) in (("A", ("ll", "hl")), ("B", ("lh", "hh"))):
                            w0 = 0 if grp == "A" else 256
                            ps = ppool.tile([128, W], F32)
                            nc.tensor.matmul(
                                out=ps,
                                lhsT=wr[:, w0 : w0 + 128],
                                rhs=xr[:, 2 * h * W : (2 * h + 1) * W],
                                start=True,
                                stop=False,
                            )
                            nc.tensor.matmul(
                                out=ps,
                                lhsT=wr[:, w0 + 128 : w0 + 256],
                                rhs=xr[:, (2 * h + 1) * W : (2 * h + 2) * W],
                                start=False,
                                stop=True,
                            )
                            od = odpool.tile([128, OW], BF16, tag="od")
                            nc.scalar.copy(out=od, in_=ps[:, 1:W:2])
                            nc.vector.tensor_add(
                                out=otile[lo][:, ms], in0=ps[:, 0:W:2], in1=od
                            )
                            # thigh = od - ev = 2*od - (ev + od), all-SBUF bf16
                            nc.vector.scalar_tensor_tensor(
                                out=otile[hi][:, ms],
                                in0=od,
                                scalar=2.0,
                                in1=otile[lo][:, ms],
                                op0=mybir.AluOpType.mult,
                                op1=mybir.AluOpType.subtract,
                            )
                for k, name in enumerate(("ll", "lh", "hl", "hh")):
                    nc.scalar.dma_start(
                        out=out4[k, c0 : c0 + G].rearrange(
                            "g (h p) j -> p (g h) j", p=128
                        ),
                        in_=otile[name][:, :].rearrange("p (m j) -> p m j", j=OW),
                    )
    nc.compile()
    return nc


def make_w():
    w = np.zeros((128, 512), np.float32)
    for q in range(64):
        w[2 * q, q] = 0.5  # vA1: P pairs -> partitions 0:64
        w[2 * q + 1, q] = 0.5
        w[2 * q, 128 + 64 + q] = 0.5  # vA2: P pairs -> partitions 64:128
        w[2 * q + 1, 128 + 64 + q] = 0.5
        w[2 * q, 256 + q] = -0.5  # vB1: M pairs -> partitions 0:64
        w[2 * q + 1, 256 + q] = 0.5
        w[2 * q, 384 + 64 + q] = -0.5  # vB2: M pairs -> partitions 64:128
        w[2 * q + 1, 384 + 64 + q] = 0.5
    return w.astype(ml_dtypes.bfloat16)


def get_nc():
    if "nc" not in _cache:
        _cache["nc"] = build_nc()
    return _cache["nc"]


def kernel(x):
    x = np.asarray(x, dtype=np.float32).astype(ml_dtypes.bfloat16)
    assert x.shape == (N_CORES, C, H, W), x.shape
    nc = get_nc()
    w = make_w()
    in_maps = [{"x": x[i], "w": w} for i in range(N_CORES)]
    res = run_bass_kernel_spmd(nc, in_maps, list(range(N_CORES)))
    full = np.stack(
        [res.results[i]["out4"].astype(np.float32) for i in range(N_CORES)], axis=0
    )
    return tuple(full[:, k] for k in range(4))



# revision 13
# speedup vs baseline: 1.0419x; 1.0419x over previous
"""Haar DWT (2x2) Trainium2 Bass kernel, v8: one matmul per 128-row block.

Full input x: (8, 64, 512, 512) fp32. Output: tuple (ll, lh, hl, hh), each
(8, 64, 256, 256) fp32. Core i processes batch element i (pure data parallel).

bf16 I/O halves HBM traffic (rel-err gate 2e-2, bf16 contributes ~6e-3):
per-core DMA = 32MB in + 32MB out + 32KB w = 186.7us at the modeled
360GB/s exclusive DMA bandwidth. That is the floor; everything else must
stay strictly below it.

v8 change vs v7: the vertical Haar stage used to stream every input block
through the PE twice (once for pair-sums, once for pair-diffs; 512 matmuls).
One [128,128] weight matrix computes both at once: w cols 0:64 produce
0.5*(row2q+row2q+1), cols 64:128 produce 0.5*(row2q+1-row2q), so each
128-row block needs ONE matmul (256 total) and PE busy drops to ~half of
the DMA floor even at the mid p-state.

Per-core design (per 512x512 channel image):
  - ONE input DMA per channel on the SP ring:
    xc[p, r*512 + j] = x[c, r*128 + p, j] (bf16, 512 descriptors x 1KB).
  - Per block r (4 per channel): matmul -> psum[128,512] fp32:
    partitions 0:64 = vertical pair-sums (rows 64r+q), 64:128 = pair-diffs.
  - Horizontal stage per psum (balanced so no engine exceeds the DMA floor):
      ACT: od = copy(ps odd cols) -> bf16 SBUF        (one PSUM read)
      DVE: tl  = ps even cols + od -> bf16            (one PSUM read, 1x)
      DVE: th  = (od * 2) - tl                        (all-SBUF bf16
           scalar_tensor_tensor, fast DVE perf mode)
    tl partitions 0:64 = ll, 64:128 = lh; th: 0:64 = hl, 64:128 = hh.
  - Stores on the ACT ring, 2 per G=2-channel group into fused bf16 DRAM
    tensor out4 [4, C, 256, 256]: one covers out4[0:2] (ll+lh), one
    out4[2:4] (hl+hh); 512B descriptors (at the full-bandwidth threshold).

DMA count per core: 1 + 64 + 64 = 129.
"""

import sys

if "/opt/trn_rl_repo" not in sys.path:
    sys.path.insert(0, "/opt/trn_rl_repo")

import ml_dtypes
import numpy as np

import concourse.mybir as mybir
from concourse.bacc import Bacc
from concourse.tile import TileContext
from concourse.bass_utils import run_bass_kernel_spmd

N_CORES = 8
C = 64  # images (channels) per core
G = 2  # channels per output-store group
H = W = 512
OH = OW = 256
F32 = mybir.dt.float32
BF16 = mybir.dt.bfloat16

_cache = {}


def build_nc():
    nc = Bacc("TRN2", target_bir_lowering=False, debug=False, num_devices=N_CORES)
    x = nc.declare_dram_parameter("x", [C, H, W], BF16, isOutput=False)
    # 256 cols (only 0:128 used) so the load's descriptors are 512B
    # (full-bandwidth threshold) instead of 256B at 2x latency.
    w = nc.declare_dram_parameter("w", [128, 256], BF16, isOutput=False)
    # outA[k, q, c, r, j] = (ll, lh)[k][c, 64*r + q, j]; outB likewise (hl, hh).
    # The q-major row layout keeps the partition dim (k q) adjacent for the
    # store AP; the host undoes the (r, q) permutation for free.
    outA = nc.declare_dram_parameter("outA", [2, 64, C, 4, OW], BF16, isOutput=True)
    outB = nc.declare_dram_parameter("outB", [2, 64, C, 4, OW], BF16, isOutput=True)

    with TileContext(nc) as tc:
        with (
            tc.tile_pool(name="const", bufs=1) as cpool,
            tc.tile_pool(name="xin", bufs=8) as xpool,
            tc.tile_pool(name="outs", bufs=4) as opool,
            tc.tile_pool(name="odbuf", bufs=8) as odpool,
            tc.tile_pool(name="psum", bufs=8, space="PSUM") as ppool,
        ):
            wt = cpool.tile([128, 256], BF16)
            wload = [False]

            def load_w():
                # Deferred until after the first x load is queued so the
                # first input transfer starts as early as possible.
                if not wload[0]:
                    nc.sync.dma_start(out=wt, in_=w[:, :])
                    wload[0] = True

            wr = wt[:, 0:128]
            for c0 in range(0, C, G):
                last_group = c0 == C - G
                if last_group:
                    # Tail: per-channel tiles + stores on the (now idle) SP
                    # queue, so the final store transfer is 728ns instead of
                    # 1456ns and earlier tail data streams out while the last
                    # channel's ACT->DVE->Pool chain finishes.
                    for gi in range(G):
                        c = c0 + gi
                        xc = xpool.tile([128, 4 * W], BF16)
                        nc.sync.dma_start(
                            out=xc[:, :].rearrange("p (r j) -> p r j", r=4),
                            in_=x[c].rearrange("(r p) j -> p r j", p=128),
                        )
                        xr = xc[:, :]
                        tlc = opool.tile([128, 4 * OW], BF16, tag="tl", name=f"tl_{c}")
                        thc = opool.tile([128, 4 * OW], BF16, tag="th", name=f"th_{c}")
                        for r in range(4):
                            ms = slice(r * OW, (r + 1) * OW)
                            ps = ppool.tile([128, W], F32)
                            nc.tensor.matmul(
                                out=ps,
                                lhsT=wr,
                                rhs=xr[:, r * W : (r + 1) * W],
                                start=True,
                                stop=True,
                            )
                            od2 = odpool.tile([128, OW], BF16, tag="od")
                            nc.scalar.activation(
                                out=od2,
                                in_=ps[:, 1:W:2],
                                func=mybir.ActivationFunctionType.Copy,
                                scale=2.0,
                            )
                            nc.vector.scalar_tensor_tensor(
                                out=tlc[:, ms],
                                in0=od2,
                                scalar=0.5,
                                in1=ps[:, 0:W:2],
                                op0=mybir.AluOpType.mult,
                                op1=mybir.AluOpType.add,
                            )
                            nc.vector.tensor_sub(
                                out=thc[:, ms], in0=od2, in1=tlc[:, ms]
                            )
                        nc.sync.dma_start(
                            out=outA[:, :, c].rearrange("k q r j -> (k q) r j"),
                            in_=tlc[:, :].rearrange("p (r j) -> p r j", j=OW),
                        )
                        nc.sync.dma_start(
                            out=outB[:, :, c].rearrange("k q r j -> (k q) r j"),
                            in_=thc[:, :].rearrange("p (r j) -> p r j", j=OW),
                        )
                    continue
                tl = opool.tile([128, G * 4 * OW], BF16, tag="tl", name=f"tl_{c0}")
                th = opool.tile([128, G * 4 * OW], BF16, tag="th", name=f"th_{c0}")
                for gi in range(G):
                    c = c0 + gi
                    xc = xpool.tile([128, 4 * W], BF16)
                    nc.sync.dma_start(
                        out=xc[:, :].rearrange("p (r j) -> p r j", r=4),
                        in_=x[c].rearrange("(r p) j -> p r j", p=128),
                    )
                    load_w()
                    xr = xc[:, :]
                    for r in range(4):
                        ms = slice((gi * 4 + r) * OW, (gi * 4 + r + 1) * OW)
                        ps = ppool.tile([128, W], F32)
                        nc.tensor.matmul(
                            out=ps,
                            lhsT=wr,
                            rhs=xr[:, r * W : (r + 1) * W],
                            start=True,
                            stop=True,
                        )
                        # Horizontal stage, 984ns of engine time per psum:
                        #   ACT: od2 = 2*odd (scale is free in the act op)
                        #   DVE: tl = 0.5*od2 + even   (PSUM operand, 1x: 392ns)
                        #   DVE: th = od2 - tl          (all-SBUF bf16, 2x: 194ns)
                        # Pool/GPSIMD cannot run compute ops on real NCv3, and
                        # two 1x DVE ops would co-saturate DVE with the DMA
                        # floor; this split keeps DVE at ~150us.
                        od2 = odpool.tile([128, OW], BF16, tag="od")
                        nc.scalar.activation(
                            out=od2,
                            in_=ps[:, 1:W:2],
                            func=mybir.ActivationFunctionType.Copy,
                            scale=2.0,
                        )
                        nc.vector.scalar_tensor_tensor(
                            out=tl[:, ms],
                            in0=od2,
                            scalar=0.5,
                            in1=ps[:, 0:W:2],
                            op0=mybir.AluOpType.mult,
                            op1=mybir.AluOpType.add,
                        )
                        nc.vector.tensor_sub(
                            out=th[:, ms], in0=od2, in1=tl[:, ms]
                        )
                nc.scalar.dma_start(
                    out=outA[:, :, c0 : c0 + G].rearrange("k q c r j -> (k q) c r j"),
                    in_=tl[:, :].rearrange("p (c r j) -> p c r j", r=4, j=OW),
                )
                nc.scalar.dma_start(
                    out=outB[:, :, c0 : c0 + G].rearrange("k q c r j -> (k q) c r j"),
                    in_=th[:, :].rearrange("p (c r j) -> p c r j", r=4, j=OW),
                )
    nc.compile()
    return nc


def make_w():
    # [128, 256]: only cols 0:128 are used (rest pads rows to 512B so the
    # w load's DMA descriptors hit the full-bandwidth threshold).
    w = np.zeros((128, 256), np.float32)
    for q in range(64):
        w[2 * q, q] = 0.5  # pair sums -> partitions 0:64
        w[2 * q + 1, q] = 0.5
        w[2 * q, 64 + q] = -0.5  # pair diffs -> partitions 64:128
        w[2 * q + 1, 64 + q] = 0.5
    return w.astype(ml_dtypes.bfloat16)


def get_nc():
    if "nc" not in _cache:
        _cache["nc"] = build_nc()
    return _cache["nc"]


def kernel(x):
    x = np.asarray(x, dtype=np.float32).astype(ml_dtypes.bfloat16)
    assert x.shape == (N_CORES, C, H, W), x.shape
    nc = get_nc()
    w = make_w()
    in_maps = [{"x": x[i], "w": w} for i in range(N_CORES)]
    res = run_bass_kernel_spmd(nc, in_maps, list(range(N_CORES)))
    outs = []
    for name, k in (("outA", 0), ("outA", 1), ("outB", 0), ("outB", 1)):
        # res[name][k] is [q, C, r, j]; output row = 64*r + q.
        full = np.stack(
            [
                res.results[i][name][k]
                .astype(np.float32)
                .transpose(1, 2, 0, 3)
                .reshape(C, OH, OW)
                for i in range(N_CORES)
            ],
            axis=0,
        )
        outs.append(full)
    return tuple(outs)
